# revision 1
# baseline (speedup 1.0000x reference)
"""ButterflyLinear TRN2 kernel — 8-core data-parallel dense matmul.

The module  out = blockdiag(shuffle(blockdiag(x, stage1)) @ mix_w.T, stage2)
is a fixed linear map on the 2048-d feature axis.  We fold
stage1 ∘ shuffle ∘ mix ∘ stage2 into a single dense A [2048, 2048] on the
host (cheap block-wise einsums, fp64), then each NeuronCore computes
yT = A.T @ xT for its 2048-token shard: feature-major layout so the
contraction dim sits on SBUF partitions.  Operands are fp16 on the device
(FWL-fast weight loads, half the DMA bytes); accumulation is fp32 PSUM and
the output is fp32.  End-to-end relative error ~7e-4.
"""

import sys

if "/opt/trn_rl_repo" not in sys.path:
    sys.path.insert(0, "/opt/trn_rl_repo")

import numpy as np

IN_F = 2048
OUT_F = 2048
BS = 64
NIB = IN_F // BS
NOB = OUT_F // BS
N_CORES = 8
TOK_PC = 2048  # tokens per core (16384 / 8)

P = 128
KT = IN_F // P  # 16 k-tiles
MT = OUT_F // P  # 16 m-tiles
NT = 512  # token tile (matmul moving dim)
NN = TOK_PC // NT  # 4 token tiles per core

_CACHE = {}


def _build(repeats: int = 1, loop_iters: int = 0):
    """Build + compile the per-core Bass program (SPMD, same on all cores).

    loop_iters > 0 wraps the body in a hardware For_i loop (timing builds)."""
    import contextlib

    import concourse.mybir as mybir
    import concourse.tile as tile
    from concourse import bacc

    nc = bacc.Bacc(None, target_bir_lowering=False, debug=False)
    f32 = mybir.dt.float32
    f16 = mybir.dt.float16

    # A is host-pretiled to [m, p, k, c] so each per-m load is 4KB-contiguous
    # per partition.  x is [in_feat, tokens] (feature-major).
    a_ext = nc.declare_dram_parameter("a", [MT, P, KT, P], f16, isOutput=False)
    x_ext = nc.declare_dram_parameter("x", [IN_F, TOK_PC], f16, isOutput=False)
    y_ext = nc.declare_dram_parameter("y", [OUT_F, TOK_PC], f32, isOutput=True)

    x_v = x_ext[:].rearrange("(k p) t -> p k t", p=P)

    with tile.TileContext(nc) as tc:
        with (
            tc.tile_pool(name="a_pool", bufs=1) as a_pool,
            tc.tile_pool(name="x_pool", bufs=3) as x_pool,
            tc.tile_pool(name="o_pool", bufs=6) as o_pool,
            tc.tile_pool(name="ps", bufs=6, space="PSUM") as ps_pool,
        ):
            loop_cm = (
                tc.For_i(0, loop_iters, 1, hint_engines=(mybir.EngineType.PE,))
                if loop_iters
                else contextlib.nullcontext()
            )
            with loop_cm:
                for _rep in range(repeats):
                    # A fully resident, loaded in m-columns so compute can
                    # start as soon as the first column lands.  Scalar-engine
                    # HWDGE ring so A loads don't head-of-line block the
                    # X-slab loads issued on the sync ring.
                    a_t = []
                    for m in range(MT):
                        at = a_pool.tile([P, KT, P], f16, tag=f"a{m}")
                        nc.scalar.dma_start(at[:], a_ext[m])
                        a_t.append(at)
                    for n in range(NN):
                        x_t = x_pool.tile([P, KT, NT], f16, tag="x")
                        for k in range(KT):
                            nc.sync.dma_start(
                                x_t[:, k, :], x_v[:, k, n * NT : (n + 1) * NT]
                            )
                        for m in range(MT):
                            ps = ps_pool.tile([P, NT], f32)
                            for k in range(KT):
                                nc.tensor.matmul(
                                    ps[:],
                                    a_t[m][:, k, :],
                                    x_t[:, k, :],
                                    start=(k == 0),
                                    stop=(k == KT - 1),
                                )
                            o_t = o_pool.tile([P, NT], f32, tag="o")
                            nc.vector.tensor_copy(o_t[:], ps[:])
                            # gpsimd SWDGE ring: output stores must not
                            # share the sync FIFO with X-slab loads
                            nc.gpsimd.dma_start(
                                y_ext[m * P : (m + 1) * P, n * NT : (n + 1) * NT],
                                o_t[:],
                            )
    nc.compile()
    return nc


def get_nc(repeats: int = 1, loop_iters: int = 0):
    key = ("nc", repeats, loop_iters)
    if key not in _CACHE:
        _CACHE[key] = _build(repeats, loop_iters)
    return _CACHE[key]


def compose_A(stage1: np.ndarray, stage2: np.ndarray, mix_w: np.ndarray) -> np.ndarray:
    """Fold stage1 ∘ shuffle ∘ mix ∘ stage2 into one dense [IN_F, OUT_F] map."""
    j = np.arange(IN_F)
    sig = (j % BS) * NIB + j // BS  # shuffle: h2[:, j] = h1[:, sig(j)]
    Ms = np.empty((IN_F, OUT_F), dtype=np.float64)
    Ms[sig, :] = mix_w.T.astype(np.float64)  # y = h1 @ Ms
    A_mid = np.einsum(
        "gcd,gdo->gco",
        stage1.reshape(NIB, BS, BS).astype(np.float64),
        Ms.reshape(NIB, BS, OUT_F),
    ).reshape(IN_F, OUT_F)
    A = np.einsum(
        "igc,gcd->igd",
        A_mid.reshape(IN_F, NOB, BS),
        stage2.reshape(NOB, BS, BS).astype(np.float64),
    ).reshape(IN_F, OUT_F)
    return A.astype(np.float32)


def tile_A(A: np.ndarray) -> np.ndarray:
    """[IN_F, OUT_F] fp32 -> [m, p, k, c] fp16 device layout."""
    return np.ascontiguousarray(
        A.reshape(KT, P, MT, P).transpose(2, 1, 0, 3).astype(np.float16)
    )


def make_in_maps(x, stage1, stage2, mix_w):
    A = compose_A(np.asarray(stage1), np.asarray(stage2), np.asarray(mix_w))
    A_dev = tile_A(A)
    x_flat = np.ascontiguousarray(np.asarray(x), dtype=np.float32).reshape(-1, IN_F)
    in_maps = []
    for c in range(N_CORES):
        shard = x_flat[c * TOK_PC : (c + 1) * TOK_PC, :]
        xT = np.ascontiguousarray(shard.T.astype(np.float16))
        in_maps.append({"a": A_dev, "x": xT})
    return in_maps


def assemble_output(results, batch_shape):
    y_flat = np.empty((N_CORES * TOK_PC, OUT_F), dtype=np.float32)
    for c in range(N_CORES):
        y_flat[c * TOK_PC : (c + 1) * TOK_PC, :] = results[c]["y"].T
    return y_flat.reshape(*batch_shape, OUT_F)


def kernel(x, stage1, stage2, mix_w):
    from concourse.bass_utils import run_bass_kernel_spmd

    batch_shape = np.asarray(x).shape[:-1]
    nc = get_nc()
    in_maps = make_in_maps(x, stage1, stage2, mix_w)
    res = run_bass_kernel_spmd(nc, in_maps, list(range(N_CORES)))
    return assemble_output(res.results, batch_shape)



# revision 4
# speedup vs baseline: 1.0547x; 1.0547x over previous
"""ButterflyLinear TRN2 kernel — 8-core data-parallel dense matmul.

The module  out = blockdiag(shuffle(blockdiag(x, stage1)) @ mix_w.T, stage2)
is a fixed linear map on the 2048-d feature axis.  We fold
stage1 ∘ shuffle ∘ mix ∘ stage2 into a single dense A [2048, 2048] on the
host (cheap block-wise einsums, fp64), then each NeuronCore computes
yT = A.T @ xT for its 2048-token shard: feature-major layout so the
contraction dim sits on SBUF partitions.  Operands are fp16 on the device
(FWL-fast weight loads, half the DMA bytes); accumulation is fp32 PSUM and
the output is fp32.

v2: the hot loop is ordered (m, k, n) so each stationary tile a[m,k] is
streamed against all G=4 moving token slabs; a post-build pass removes the
redundant per-matmul InstLdweights (the add-time split emits one per
matmul even when the weights AP is unchanged), cutting PE weight loads
4x.  x is double-buffered so back-to-back kernel iterations overlap.
"""

import sys

if "/opt/trn_rl_repo" not in sys.path:
    sys.path.insert(0, "/opt/trn_rl_repo")

import numpy as np

IN_F = 2048
OUT_F = 2048
BS = 64
NIB = IN_F // BS
NOB = OUT_F // BS
N_CORES = 8
TOK_PC = 2048  # tokens per core (16384 / 8)

P = 128
KT = IN_F // P  # 16 k-tiles
MT = OUT_F // P  # 16 m-tiles
NT = 512  # token tile (matmul moving dim)
NN = TOK_PC // NT  # 4 token tiles per core

_CACHE = {}


def _ap_key(pap):
    try:
        return (pap.memref, pap.offset, str(pap.ap), str(pap.dtype))
    except Exception:
        return None


def dedup_ldweights(nc):
    """Remove InstLdweights whose weights AP matches the previous PE weight
    load with no intervening clobber.  Matmuls were already split into
    (InstLdweights, InstMatmult(ldweights=False)) pairs at add time, so the
    later duplicates are pure overhead; dangling dependency references are
    remapped onto the duplicate's own matmul (same engine, later in program
    order — conservative for WAR on the SBUF weights tile)."""
    import concourse.mybir as mybir

    removed_total = 0
    pe = mybir.EngineType.PE
    for fn in nc.m.functions:
        for blk in fn.blocks:
            insts = list(blk.instructions)
            keep, removed_map = [], {}
            prev_key = None
            for idx, inst in enumerate(insts):
                tn = type(inst).__name__
                if tn == "InstLdweights":
                    key = _ap_key(inst.ins[0])
                    nxt = None
                    for j in range(idx + 1, len(insts)):
                        if insts[j].engine == pe:
                            nxt = insts[j]
                            break
                    if (
                        key is not None
                        and key == prev_key
                        and nxt is not None
                        and type(nxt).__name__ == "InstMatmult"
                        and not nxt.is_transpose
                    ):
                        nxt.merge_dependencies_from(inst)
                        removed_map[inst.name] = nxt.name
                        continue
                    prev_key = key
                elif inst.engine == pe:
                    if tn == "InstMatmult" and not inst.is_transpose:
                        pass  # plain matmul does not clobber loaded weights
                    elif tn in ("InstEventSemaphore", "InstDrain", "InstNop"):
                        pass  # sequencer-only
                    else:
                        prev_key = None
            # rebuild the list only if something was removed
            if removed_map:
                keep = [i for i in insts if i.name not in removed_map]
                blk.instructions = keep
                removed_total += len(removed_map)
                for fn2 in nc.m.functions:
                    for blk2 in fn2.blocks:
                        for inst2 in blk2.instructions:
                            inst2.remap_dependency_names(removed_map)
    return removed_total


def _build(repeats: int = 1, loop_iters: int = 0):
    """Build + compile the per-core Bass program (SPMD, same on all cores).

    loop_iters > 0 wraps the body in a hardware For_i loop (timing builds)."""
    import contextlib

    import concourse.mybir as mybir
    import concourse.tile as tile
    from concourse import bacc

    nc = bacc.Bacc(None, target_bir_lowering=False, debug=False)
    f32 = mybir.dt.float32
    f16 = mybir.dt.float16

    # A is host-pretiled to [m, p, k, c] so each per-m load is 4KB-contiguous
    # per partition.  x is [in_feat, tokens] (feature-major).
    a_ext = nc.declare_dram_parameter("a", [MT, P, KT, P], f16, isOutput=False)
    x_ext = nc.declare_dram_parameter("x", [IN_F, TOK_PC], f16, isOutput=False)
    y_ext = nc.declare_dram_parameter("y", [OUT_F, TOK_PC], f32, isOutput=True)

    x_v = x_ext[:].rearrange("(k p) t -> p k t", p=P)

    with tile.TileContext(nc) as tc:
        with (
            tc.tile_pool(name="a_pool", bufs=1) as a_pool,
            tc.tile_pool(name="x_pool", bufs=2) as x_pool,
            tc.tile_pool(name="o_pool", bufs=4) as o_pool,
            tc.tile_pool(name="ps", bufs=8, space="PSUM") as ps_pool,
        ):
            loop_cm = (
                tc.For_i(0, loop_iters, 1, hint_engines=(mybir.EngineType.PE,))
                if loop_iters
                else contextlib.nullcontext()
            )
            with loop_cm:
                for _rep in range(repeats):
                    # A on the scalar HWDGE ring (per-m tiles), x k-major on
                    # the sync HWDGE ring so the first k-slabs land fast.
                    a_t = []
                    for m in range(MT):
                        at = a_pool.tile([P, KT, P], f16, tag=f"a{m}")
                        nc.scalar.dma_start(at[:], a_ext[m])
                        a_t.append(at)
                    x_t = []
                    for n in range(NN):
                        xt = x_pool.tile([P, KT, NT], f16, tag=f"x{n}")
                        x_t.append(xt)
                    for k in range(KT):
                        for n in range(NN):
                            nc.sync.dma_start(
                                x_t[n][:, k, :], x_v[:, k, n * NT : (n + 1) * NT]
                            )
                    for m in range(MT):
                        ps = []
                        for _n in range(NN):
                            pst = ps_pool.tile([P, NT], f32, tag="ps")
                            ps.append(pst)
                        for k in range(KT):
                            for n in range(NN):
                                nc.tensor.matmul(
                                    ps[n][:],
                                    a_t[m][:, k, :],
                                    x_t[n][:, k, :],
                                    start=(k == 0),
                                    stop=(k == KT - 1),
                                )
                        for n in range(NN):
                            o_t = o_pool.tile([P, NT], f32, tag="o")
                            nc.vector.tensor_copy(o_t[:], ps[n][:])
                            # gpsimd SWDGE ring: output stores must not
                            # share the sync FIFO with X-slab loads
                            nc.gpsimd.dma_start(
                                y_ext[m * P : (m + 1) * P, n * NT : (n + 1) * NT],
                                o_t[:],
                            )
    n_removed = dedup_ldweights(nc)
    assert n_removed >= repeats * MT * KT * (NN - 1) // 2, n_removed
    nc.compile()
    return nc


def get_nc(repeats: int = 1, loop_iters: int = 0):
    key = ("nc", repeats, loop_iters)
    if key not in _CACHE:
        _CACHE[key] = _build(repeats, loop_iters)
    return _CACHE[key]


def compose_A(stage1: np.ndarray, stage2: np.ndarray, mix_w: np.ndarray) -> np.ndarray:
    """Fold stage1 ∘ shuffle ∘ mix ∘ stage2 into one dense [IN_F, OUT_F] map."""
    j = np.arange(IN_F)
    sig = (j % BS) * NIB + j // BS  # shuffle: h2[:, j] = h1[:, sig(j)]
    Ms = np.empty((IN_F, OUT_F), dtype=np.float64)
    Ms[sig, :] = mix_w.T.astype(np.float64)  # y = h1 @ Ms
    A_mid = np.einsum(
        "gcd,gdo->gco",
        stage1.reshape(NIB, BS, BS).astype(np.float64),
        Ms.reshape(NIB, BS, OUT_F),
    ).reshape(IN_F, OUT_F)
    A = np.einsum(
        "igc,gcd->igd",
        A_mid.reshape(IN_F, NOB, BS),
        stage2.reshape(NOB, BS, BS).astype(np.float64),
    ).reshape(IN_F, OUT_F)
    return A.astype(np.float32)


def tile_A(A: np.ndarray) -> np.ndarray:
    """[IN_F, OUT_F] fp32 -> [m, p, k, c] fp16 device layout."""
    return np.ascontiguousarray(
        A.reshape(KT, P, MT, P).transpose(2, 1, 0, 3).astype(np.float16)
    )


def make_in_maps(x, stage1, stage2, mix_w):
    A = compose_A(np.asarray(stage1), np.asarray(stage2), np.asarray(mix_w))
    A_dev = tile_A(A)
    x_flat = np.ascontiguousarray(np.asarray(x), dtype=np.float32).reshape(-1, IN_F)
    in_maps = []
    for c in range(N_CORES):
        shard = x_flat[c * TOK_PC : (c + 1) * TOK_PC, :]
        xT = np.ascontiguousarray(shard.T.astype(np.float16))
        in_maps.append({"a": A_dev, "x": xT})
    return in_maps


def assemble_output(results, batch_shape):
    y_flat = np.empty((N_CORES * TOK_PC, OUT_F), dtype=np.float32)
    for c in range(N_CORES):
        y_flat[c * TOK_PC : (c + 1) * TOK_PC, :] = results[c]["y"].T
    return y_flat.reshape(*batch_shape, OUT_F)


def kernel(x, stage1, stage2, mix_w):
    from concourse.bass_utils import run_bass_kernel_spmd

    batch_shape = np.asarray(x).shape[:-1]
    nc = get_nc()
    in_maps = make_in_maps(x, stage1, stage2, mix_w)
    res = run_bass_kernel_spmd(nc, in_maps, list(range(N_CORES)))
    return assemble_output(res.results, batch_shape)


# revision 16
# speedup vs baseline: 1.0579x; 1.0031x over previous
"""ButterflyLinear TRN2 kernel — 8-core data-parallel dense matmul.

The module  out = blockdiag(shuffle(blockdiag(x, stage1)) @ mix_w.T, stage2)
is a fixed linear map on the 2048-d feature axis.  We fold
stage1 ∘ shuffle ∘ mix ∘ stage2 into a single dense A [2048, 2048] on the
host (cheap block-wise einsums, fp64), then each NeuronCore computes
yT = A.T @ xT for its 2048-token shard: feature-major layout so the
contraction dim sits on SBUF partitions.  Operands are fp16 on the device
(FWL-fast weight loads, half the DMA bytes); accumulation is fp32 PSUM and
the output is fp32.

v2: the hot loop is ordered (m, k, n) so each stationary tile a[m,k] is
streamed against all G=4 moving token slabs; a post-build pass removes the
redundant per-matmul InstLdweights (the add-time split emits one per
matmul even when the weights AP is unchanged), cutting PE weight loads
4x.  x is double-buffered so back-to-back kernel iterations overlap.
"""

import sys

if "/opt/trn_rl_repo" not in sys.path:
    sys.path.insert(0, "/opt/trn_rl_repo")

import numpy as np

IN_F = 2048
OUT_F = 2048
BS = 64
NIB = IN_F // BS
NOB = OUT_F // BS
N_CORES = 8
TOK_PC = 2048  # tokens per core (16384 / 8)

P = 128
KT = IN_F // P  # 16 k-tiles
MT = OUT_F // P  # 16 m-tiles
NT = 512  # token tile (matmul moving dim)
NN = TOK_PC // NT  # 4 token tiles per core

_CACHE = {}


def _ap_key(pap):
    try:
        return (pap.memref, pap.offset, str(pap.ap), str(pap.dtype))
    except Exception:
        return None


def dedup_ldweights(nc):
    """Remove InstLdweights whose weights AP matches the previous PE weight
    load with no intervening clobber.  Matmuls were already split into
    (InstLdweights, InstMatmult(ldweights=False)) pairs at add time, so the
    later duplicates are pure overhead; dangling dependency references are
    remapped onto the duplicate's own matmul (same engine, later in program
    order — conservative for WAR on the SBUF weights tile)."""
    import concourse.mybir as mybir

    removed_total = 0
    pe = mybir.EngineType.PE
    for fn in nc.m.functions:
        for blk in fn.blocks:
            insts = list(blk.instructions)
            keep, removed_map = [], {}
            prev_key = None
            for idx, inst in enumerate(insts):
                tn = type(inst).__name__
                if tn == "InstLdweights":
                    key = _ap_key(inst.ins[0])
                    nxt = None
                    for j in range(idx + 1, len(insts)):
                        if insts[j].engine == pe:
                            nxt = insts[j]
                            break
                    if (
                        key is not None
                        and key == prev_key
                        and nxt is not None
                        and type(nxt).__name__ == "InstMatmult"
                        and not nxt.is_transpose
                    ):
                        nxt.merge_dependencies_from(inst)
                        removed_map[inst.name] = nxt.name
                        continue
                    prev_key = key
                elif inst.engine == pe:
                    if tn == "InstMatmult" and not inst.is_transpose:
                        pass  # plain matmul does not clobber loaded weights
                    elif tn in ("InstEventSemaphore", "InstDrain", "InstNop"):
                        pass  # sequencer-only
                    else:
                        prev_key = None
            # rebuild the list only if something was removed
            if removed_map:
                keep = [i for i in insts if i.name not in removed_map]
                blk.instructions = keep
                removed_total += len(removed_map)
                for fn2 in nc.m.functions:
                    for blk2 in fn2.blocks:
                        for inst2 in blk2.instructions:
                            inst2.remap_dependency_names(removed_map)
    return removed_total


def _build(
    repeats: int = 1,
    loop_iters: int = 0,
    hoist_in_dma: bool = False,
    drop_y: bool = False,
    drop_copy: bool = False,
):
    """Build + compile the per-core Bass program (SPMD, same on all cores).

    loop_iters > 0 wraps the body in a hardware For_i loop (timing builds).
    hoist_in_dma/drop_y/drop_copy are perf-bisection knobs (timing only)."""
    import contextlib

    import concourse.mybir as mybir
    import concourse.tile as tile
    from concourse import bacc

    nc = bacc.Bacc(None, target_bir_lowering=False, debug=False)
    f32 = mybir.dt.float32
    f16 = mybir.dt.float16
    bf16 = mybir.dt.bfloat16

    # A is host-pretiled to [m, p, k, c] so each per-m load is 4KB-contiguous
    # per partition.  x is [in_feat, tokens] (feature-major).  y is stored
    # bf16 (host upcasts) to halve the output DMA traffic.
    a_ext = nc.declare_dram_parameter("a", [MT, P, KT, P], f16, isOutput=False)
    x_ext = nc.declare_dram_parameter("x", [IN_F, TOK_PC], f16, isOutput=False)
    y_ext = nc.declare_dram_parameter("y", [OUT_F, TOK_PC], bf16, isOutput=True)

    x_v = x_ext[:].rearrange("(k p) t -> p k t", p=P)

    with tile.TileContext(nc) as tc:
        with (
            tc.tile_pool(name="a_pool", bufs=1) as a_pool,
            tc.tile_pool(name="x_pool", bufs=2) as x_pool,
            tc.tile_pool(name="o_pool", bufs=3) as o_pool,
            tc.tile_pool(name="ps", bufs=2, space="PSUM") as ps_pool,
        ):
            def load_inputs():
                # A on the scalar HWDGE ring (per-m tiles), x one strided
                # DMA per token slab on the sync HWDGE ring.
                a_t = []
                for m in range(MT):
                    at = a_pool.tile([P, KT, P], f16, tag=f"a{m}")
                    nc.scalar.dma_start(at[:], a_ext[m])
                    a_t.append(at)
                x_t = []
                XC = 4  # k-chunks per slab DMA
                for n in range(NN):
                    xt = x_pool.tile([P, KT, NT], f16, tag=f"x{n}")
                    for c in range(XC):
                        ks = KT // XC
                        nc.sync.dma_start(
                            xt[:, c * ks : (c + 1) * ks, :],
                            x_v[:, c * ks : (c + 1) * ks, n * NT : (n + 1) * NT],
                        )
                    x_t.append(xt)
                return a_t, x_t

            if hoist_in_dma:
                a_t, x_t = load_inputs()
            loop_cm = (
                tc.For_i(0, loop_iters, 1, hint_engines=(mybir.EngineType.PE,))
                if loop_iters
                else contextlib.nullcontext()
            )
            with loop_cm:
                for _rep in range(repeats):
                    if not hoist_in_dma:
                        a_t, x_t = load_inputs()
                    for m in range(MT):
                        # one 4-bank PSUM tile per m; each matmul writes a
                        # bank-aligned 512-col slice
                        ps = ps_pool.tile([P, NN * NT], f32, tag="ps")
                        if m == 0 and not hoist_in_dma:
                            # n-outer: stream slab 0 while slabs 1-3 land
                            order = [(k, n) for n in range(NN) for k in range(KT)]
                        else:
                            order = [(k, n) for k in range(KT) for n in range(NN)]
                        for k, n in order:
                            nc.tensor.matmul(
                                ps[:, n * NT : (n + 1) * NT],
                                a_t[m][:, k, :],
                                x_t[n][:, k, :],
                                start=(k == 0),
                                stop=(k == KT - 1),
                            )
                        if drop_copy:
                            continue
                        o_t = o_pool.tile([P, NN * NT], bf16, tag="o")
                        nc.vector.tensor_copy(o_t[:], ps[:])
                        if drop_y:
                            continue
                        # gpsimd SWDGE ring: output stores must not share
                        # the sync FIFO with X-slab loads
                        nc.gpsimd.dma_start(
                            y_ext[m * P : (m + 1) * P, :], o_t[:]
                        )
    n_removed = dedup_ldweights(nc)
    assert n_removed >= repeats * MT * KT * (NN - 1) // 2, n_removed
    nc.compile()
    return nc


def get_nc(repeats: int = 1, loop_iters: int = 0, **kw):
    key = ("nc", repeats, loop_iters, tuple(sorted(kw.items())))
    if key not in _CACHE:
        _CACHE[key] = _build(repeats, loop_iters, **kw)
    return _CACHE[key]


def compose_A(stage1: np.ndarray, stage2: np.ndarray, mix_w: np.ndarray) -> np.ndarray:
    """Fold stage1 ∘ shuffle ∘ mix ∘ stage2 into one dense [IN_F, OUT_F] map."""
    j = np.arange(IN_F)
    sig = (j % BS) * NIB + j // BS  # shuffle: h2[:, j] = h1[:, sig(j)]
    Ms = np.empty((IN_F, OUT_F), dtype=np.float64)
    Ms[sig, :] = mix_w.T.astype(np.float64)  # y = h1 @ Ms
    A_mid = np.einsum(
        "gcd,gdo->gco",
        stage1.reshape(NIB, BS, BS).astype(np.float64),
        Ms.reshape(NIB, BS, OUT_F),
    ).reshape(IN_F, OUT_F)
    A = np.einsum(
        "igc,gcd->igd",
        A_mid.reshape(IN_F, NOB, BS),
        stage2.reshape(NOB, BS, BS).astype(np.float64),
    ).reshape(IN_F, OUT_F)
    return A.astype(np.float32)


def tile_A(A: np.ndarray) -> np.ndarray:
    """[IN_F, OUT_F] fp32 -> [m, p, k, c] fp16 device layout."""
    return np.ascontiguousarray(
        A.reshape(KT, P, MT, P).transpose(2, 1, 0, 3).astype(np.float16)
    )


def make_in_maps(x, stage1, stage2, mix_w):
    A = compose_A(np.asarray(stage1), np.asarray(stage2), np.asarray(mix_w))
    A_dev = tile_A(A)
    x_flat = np.ascontiguousarray(np.asarray(x), dtype=np.float32).reshape(-1, IN_F)
    in_maps = []
    for c in range(N_CORES):
        shard = x_flat[c * TOK_PC : (c + 1) * TOK_PC, :]
        xT = np.ascontiguousarray(shard.T.astype(np.float16))
        in_maps.append({"a": A_dev, "x": xT})
    return in_maps


def assemble_output(results, batch_shape):
    y_flat = np.empty((N_CORES * TOK_PC, OUT_F), dtype=np.float32)
    for c in range(N_CORES):
        y_flat[c * TOK_PC : (c + 1) * TOK_PC, :] = results[c]["y"].T.astype(
            np.float32
        )
    return y_flat.reshape(*batch_shape, OUT_F)


def kernel(x, stage1, stage2, mix_w):
    from concourse.bass_utils import run_bass_kernel_spmd

    batch_shape = np.asarray(x).shape[:-1]
    nc = get_nc()
    in_maps = make_in_maps(x, stage1, stage2, mix_w)
    res = run_bass_kernel_spmd(nc, in_maps, list(range(N_CORES)))
    return assemble_output(res.results, batch_shape)


# revision 20
# speedup vs baseline: 1.0775x; 1.0186x over previous
"""ButterflyLinear TRN2 kernel — 8-core data-parallel dense matmul.

The module  out = blockdiag(shuffle(blockdiag(x, stage1)) @ mix_w.T, stage2)
is a fixed linear map on the 2048-d feature axis.  We fold
stage1 ∘ shuffle ∘ mix ∘ stage2 into a single dense A [2048, 2048] on the
host (cheap block-wise einsums, fp64), then each NeuronCore computes
yT = A.T @ xT for its 2048-token shard: feature-major layout so the
contraction dim sits on SBUF partitions.  Operands are fp16 on the device
(FWL-fast weight loads, half the DMA bytes); accumulation is fp32 PSUM and
the output is fp32.

v2: the hot loop is ordered (m, k, n) so each stationary tile a[m,k] is
streamed against all G=4 moving token slabs; a post-build pass removes the
redundant per-matmul InstLdweights (the add-time split emits one per
matmul even when the weights AP is unchanged), cutting PE weight loads
4x.  x is double-buffered so back-to-back kernel iterations overlap.
"""

import sys

if "/opt/trn_rl_repo" not in sys.path:
    sys.path.insert(0, "/opt/trn_rl_repo")

import numpy as np

IN_F = 2048
OUT_F = 2048
BS = 64
NIB = IN_F // BS
NOB = OUT_F // BS
N_CORES = 8
TOK_PC = 2048  # tokens per core (16384 / 8)

P = 128
KT = IN_F // P  # 16 k-tiles
MT = OUT_F // P  # 16 m-tiles
NT = 512  # token tile (matmul moving dim)
NN = TOK_PC // NT  # 4 token tiles per core

_CACHE = {}


def _ap_key(pap):
    try:
        return (pap.memref, pap.offset, str(pap.ap), str(pap.dtype))
    except Exception:
        return None


def dedup_ldweights(nc):
    """Remove InstLdweights whose weights AP matches the previous PE weight
    load with no intervening clobber.  Matmuls were already split into
    (InstLdweights, InstMatmult(ldweights=False)) pairs at add time, so the
    later duplicates are pure overhead; dangling dependency references are
    remapped onto the duplicate's own matmul (same engine, later in program
    order — conservative for WAR on the SBUF weights tile)."""
    import concourse.mybir as mybir

    removed_total = 0
    pe = mybir.EngineType.PE
    for fn in nc.m.functions:
        for blk in fn.blocks:
            insts = list(blk.instructions)
            keep, removed_map = [], {}
            prev_key = None
            for idx, inst in enumerate(insts):
                tn = type(inst).__name__
                if tn == "InstLdweights":
                    key = _ap_key(inst.ins[0])
                    nxt = None
                    for j in range(idx + 1, len(insts)):
                        if insts[j].engine == pe:
                            nxt = insts[j]
                            break
                    if (
                        key is not None
                        and key == prev_key
                        and nxt is not None
                        and type(nxt).__name__ == "InstMatmult"
                        and not nxt.is_transpose
                    ):
                        nxt.merge_dependencies_from(inst)
                        removed_map[inst.name] = nxt.name
                        continue
                    prev_key = key
                elif inst.engine == pe:
                    if tn == "InstMatmult" and not inst.is_transpose:
                        pass  # plain matmul does not clobber loaded weights
                    elif tn in ("InstEventSemaphore", "InstDrain", "InstNop"):
                        pass  # sequencer-only
                    else:
                        prev_key = None
            # rebuild the list only if something was removed
            if removed_map:
                keep = [i for i in insts if i.name not in removed_map]
                blk.instructions = keep
                removed_total += len(removed_map)
                for fn2 in nc.m.functions:
                    for blk2 in fn2.blocks:
                        for inst2 in blk2.instructions:
                            inst2.remap_dependency_names(removed_map)
    return removed_total


def _build(
    repeats: int = 1,
    loop_iters: int = 0,
    hoist_in_dma: bool = False,
    drop_y: bool = False,
    drop_copy: bool = False,
):
    """Build + compile the per-core Bass program (SPMD, same on all cores).

    loop_iters > 0 wraps the body in a hardware For_i loop (timing builds).
    hoist_in_dma/drop_y/drop_copy are perf-bisection knobs (timing only)."""
    import contextlib

    import concourse.mybir as mybir
    import concourse.tile as tile
    from concourse import bacc

    nc = bacc.Bacc(None, target_bir_lowering=False, debug=False)
    f32 = mybir.dt.float32
    f16 = mybir.dt.float16
    bf16 = mybir.dt.bfloat16

    # A is host-pretiled to [m, p, k, c] so each per-m load is 4KB-contiguous
    # per partition.  x is [in_feat, tokens] (feature-major).  y is stored
    # bf16 (host upcasts) to halve the output DMA traffic.
    a_ext = nc.declare_dram_parameter("a", [MT, P, KT, P], f16, isOutput=False)
    x_ext = nc.declare_dram_parameter("x", [IN_F, TOK_PC], f16, isOutput=False)
    y_ext = nc.declare_dram_parameter("y", [OUT_F, TOK_PC], bf16, isOutput=True)

    x_v = x_ext[:].rearrange("(k p) t -> p k t", p=P)

    with tile.TileContext(nc) as tc:
        with (
            tc.tile_pool(name="a_pool", bufs=1) as a_pool,
            tc.tile_pool(name="x_pool", bufs=2) as x_pool,
            tc.tile_pool(name="o_pool", bufs=3) as o_pool,
            tc.tile_pool(name="ps", bufs=2, space="PSUM") as ps_pool,
        ):
            def load_inputs():
                # Cold-start critical path: the PE consumes one k-group
                # (all 4 slabs at k) every ~0.9-1µs, so x streams k-major
                # across BOTH HWDGE rings (slabs 0/1 on sync, 2/3 on
                # scalar).  A rides the scalar ring: tile 0 first (needed
                # at t=0), the rest after x since tile m isn't needed
                # until m k-loops in.
                a_t = [
                    a_pool.tile([P, KT, P], f16, tag=f"a{m}", name=f"at{m}")
                    for m in range(MT)
                ]
                x_t = [
                    x_pool.tile([P, KT, NT], f16, tag=f"x{n}", name=f"xt{n}")
                    for n in range(NN)
                ]
                nc.scalar.dma_start(a_t[0][:], a_ext[0])
                for k in range(KT):
                    for n in range(NN):
                        eng = nc.sync if n < 2 else nc.scalar
                        eng.dma_start(
                            x_t[n][:, k, :], x_v[:, k, n * NT : (n + 1) * NT]
                        )
                # balance the remaining A tiles across both rings; their
                # deadlines (tile m by m k-loops in) are loose
                for m in range(1, MT):
                    eng = nc.sync if m < 8 else nc.scalar
                    eng.dma_start(a_t[m][:], a_ext[m])
                return a_t, x_t

            if hoist_in_dma:
                a_t, x_t = load_inputs()
            loop_cm = (
                tc.For_i(0, loop_iters, 1, hint_engines=(mybir.EngineType.PE,))
                if loop_iters
                else contextlib.nullcontext()
            )
            with loop_cm:
                for _rep in range(repeats):
                    if not hoist_in_dma:
                        a_t, x_t = load_inputs()
                    for m in range(MT):
                        # one 4-bank PSUM tile per m; each matmul writes a
                        # bank-aligned 512-col slice
                        ps = ps_pool.tile([P, NN * NT], f32, tag="ps")
                        for k, n in [(k, n) for k in range(KT) for n in range(NN)]:
                            nc.tensor.matmul(
                                ps[:, n * NT : (n + 1) * NT],
                                a_t[m][:, k, :],
                                x_t[n][:, k, :],
                                start=(k == 0),
                                stop=(k == KT - 1),
                            )
                        if drop_copy:
                            continue
                        o_t = o_pool.tile([P, NN * NT], bf16, tag="o")
                        nc.vector.tensor_copy(o_t[:], ps[:])
                        if drop_y:
                            continue
                        # gpsimd SWDGE ring: output stores must not share
                        # the sync FIFO with X-slab loads
                        nc.gpsimd.dma_start(
                            y_ext[m * P : (m + 1) * P, :], o_t[:]
                        )
    n_removed = dedup_ldweights(nc)
    assert n_removed >= repeats * MT * KT * (NN - 1) // 2, n_removed
    nc.compile()
    return nc


def get_nc(repeats: int = 1, loop_iters: int = 0, **kw):
    key = ("nc", repeats, loop_iters, tuple(sorted(kw.items())))
    if key not in _CACHE:
        _CACHE[key] = _build(repeats, loop_iters, **kw)
    return _CACHE[key]


def compose_A(stage1: np.ndarray, stage2: np.ndarray, mix_w: np.ndarray) -> np.ndarray:
    """Fold stage1 ∘ shuffle ∘ mix ∘ stage2 into one dense [IN_F, OUT_F] map."""
    j = np.arange(IN_F)
    sig = (j % BS) * NIB + j // BS  # shuffle: h2[:, j] = h1[:, sig(j)]
    Ms = np.empty((IN_F, OUT_F), dtype=np.float64)
    Ms[sig, :] = mix_w.T.astype(np.float64)  # y = h1 @ Ms
    A_mid = np.einsum(
        "gcd,gdo->gco",
        stage1.reshape(NIB, BS, BS).astype(np.float64),
        Ms.reshape(NIB, BS, OUT_F),
    ).reshape(IN_F, OUT_F)
    A = np.einsum(
        "igc,gcd->igd",
        A_mid.reshape(IN_F, NOB, BS),
        stage2.reshape(NOB, BS, BS).astype(np.float64),
    ).reshape(IN_F, OUT_F)
    return A.astype(np.float32)


def tile_A(A: np.ndarray) -> np.ndarray:
    """[IN_F, OUT_F] fp32 -> [m, p, k, c] fp16 device layout."""
    return np.ascontiguousarray(
        A.reshape(KT, P, MT, P).transpose(2, 1, 0, 3).astype(np.float16)
    )


def make_in_maps(x, stage1, stage2, mix_w):
    A = compose_A(np.asarray(stage1), np.asarray(stage2), np.asarray(mix_w))
    A_dev = tile_A(A)
    x_flat = np.ascontiguousarray(np.asarray(x), dtype=np.float32).reshape(-1, IN_F)
    in_maps = []
    for c in range(N_CORES):
        shard = x_flat[c * TOK_PC : (c + 1) * TOK_PC, :]
        xT = np.ascontiguousarray(shard.T.astype(np.float16))
        in_maps.append({"a": A_dev, "x": xT})
    return in_maps


def assemble_output(results, batch_shape):
    y_flat = np.empty((N_CORES * TOK_PC, OUT_F), dtype=np.float32)
    for c in range(N_CORES):
        y_flat[c * TOK_PC : (c + 1) * TOK_PC, :] = results[c]["y"].T.astype(
            np.float32
        )
    return y_flat.reshape(*batch_shape, OUT_F)


def kernel(x, stage1, stage2, mix_w):
    from concourse.bass_utils import run_bass_kernel_spmd

    batch_shape = np.asarray(x).shape[:-1]
    nc = get_nc()
    in_maps = make_in_maps(x, stage1, stage2, mix_w)
    res = run_bass_kernel_spmd(nc, in_maps, list(range(N_CORES)))
    return assemble_output(res.results, batch_shape)
